# revision 6
# baseline (speedup 1.0000x reference)
"""Trainium2 Bass kernel: topk-masked pseudo-diagonal linear layer.

Math:  a = dykstra_topk(alpha);  W[r,c] = a[(r-c)%n] * V[(r-c)%n, c];
       out = x @ W.T,   with n = 8192, x [1024, 8192], V [8192, 8192].

Strategy (8 NeuronCores, SPMD, no collectives):
  - 2D shard: 4-way over out-features r (R=2048/core) x 2-way over batch
    (BB=512/core).  Each core computes out[b0:b0+512, r0:r0+2048].
  - Host synthesizes the sharded W.T band directly (Dykstra projection is
    a tiny 8192-vector fixed-point loop; the mask is multiplied into V
    during the band gather) and ships it in bf16:
        B[c, j] = a[(r0+j-c)%n] * V[(r0+j-c)%n, c]      [n, R] bf16
    plus the per-core x.T slice in bf16.
  - Device is a pure dense matmul out = x @ B streaming B from HBM with
    x.T resident in SBUF.  bf16 operands stream the PE at 1 col/cycle
    (2.4 GHz warm) vs ~0.5 for fp32r, so the kernel is PE-bound at
    ~218 us/core with DMA (~44 MiB/core) fully hidden.
"""

import math
import numpy as np

# ---- problem constants (hardcoded; must match reference.py) ----
N = 8192
BATCH = 1024
KTOP = math.ceil((1.0 - 0.9) * N * N / N)  # 820
LR = 0.05
ITERS = 50

# ---- sharding / tiling config ----
CFG_FULL = dict(N=N, BB=512, R=2048, TN=512)


def build_nc(cfg=CFG_FULL):
    """Build + compile the single-core SPMD Bass program."""
    import concourse.bass as bass
    import concourse.tile as tile
    from concourse import bacc, mybir

    f32 = mybir.dt.float32
    bf16 = mybir.dt.bfloat16

    n, bb, r_sh, tn = cfg["N"], cfg["BB"], cfg["R"], cfg["TN"]
    rhalf = r_sh // 2
    nct = n // 128          # c-tiles (64)
    nbt = bb // 128         # b-tiles (4)
    nrt = rhalf // tn       # r-subtiles per half (2)
    assert nbt * nrt <= 8

    nc = bacc.Bacc(
        "TRN2", target_bir_lowering=False, debug=False, enable_asserts=False
    )
    xt_in = nc.dram_tensor("xt_in", [n, bb], bf16, kind="ExternalInput").ap()
    bt_in = nc.dram_tensor("bt_in", [n, r_sh], bf16, kind="ExternalInput").ap()
    out_d = nc.dram_tensor("out_d", [bb, r_sh], f32, kind="ExternalOutput").ap()

    with tile.TileContext(nc) as tc:
        with (
            tc.tile_pool(name="xt", bufs=1) as xtp,
            tc.tile_pool(name="bt", bufs=8) as btp,
            tc.tile_pool(name="ps", bufs=8, space=bass.MemorySpace.PSUM) as psp,
            tc.tile_pool(name="st", bufs=4) as stp,
        ):
            # ---- resident xT load, chunked so the first matmuls can start
            # after ~1/8 of the transfer (vector ring; B tiles use others) ----
            xt_sb = xtp.tile([128, nct, bb], bf16)
            for ch in range(8):
                nc.gpsimd.dma_start(
                    xt_sb[:, 8 * ch : 8 * (ch + 1), :],
                    xt_in[1024 * ch : 1024 * (ch + 1), :].rearrange(
                        "(ct p) b -> p ct b", p=128
                    ),
                )

            # ---- main loop: stream B tiles, matmul into 8 PSUM banks ----
            rings = [nc.sync, nc.scalar]
            for h in range(2):
                ps_tiles = [
                    psp.tile([128, tn], f32, tag="mm", name=f"ps_{h}_{i}")
                    for i in range(nbt * nrt)
                ]
                for ct in range(nct):
                    b_t = btp.tile([128, rhalf], bf16, tag="bt")
                    rings[ct % 2].dma_start(
                        b_t[:],
                        bt_in[128 * ct : 128 * (ct + 1), rhalf * h : rhalf * (h + 1)],
                    )
                    for bt in range(nbt):
                        lhsT = xt_sb[:, ct, 128 * bt : 128 * (bt + 1)]
                        for rt in range(nrt):
                            nc.tensor.matmul(
                                ps_tiles[bt * nrt + rt][:],
                                lhsT,
                                b_t[:, tn * rt : tn * (rt + 1)],
                                start=(ct == 0),
                                stop=(ct == nct - 1),
                            )
                for bt in range(nbt):
                    for rt in range(nrt):
                        st_t = stp.tile([128, tn], f32, tag="st")
                        nc.scalar.copy(st_t[:], ps_tiles[bt * nrt + rt][:])
                        nc.gpsimd.dma_start(
                            out_d[
                                128 * bt : 128 * (bt + 1),
                                rhalf * h + tn * rt : rhalf * h + tn * (rt + 1),
                            ],
                            st_t[:],
                        )
    nc.compile()
    return nc


# ---------------- host-side prep / gather ----------------

def dykstra_mask(alpha, k=KTOP, l=LR, num_iter=ITERS):
    """Euclidean projection of alpha/l onto {0 <= p <= 1, sum(p) = k}."""
    v = alpha.astype(np.float64) / l
    n = v.shape[0]
    p = np.zeros_like(v)
    q = np.zeros_like(v)
    for _ in range(num_iter):
        t = v + p
        y = t + (k - t.sum()) / n
        p = t - y
        v = np.clip(y + q, 0.0, 1.0)
        q = y + q - v
    return v.astype(np.float32)


def host_prep(x, V, alpha, cfg=CFG_FULL):
    """Build the 8 per-core input maps. Core id = ib*4 + ir."""
    import ml_dtypes

    bf16 = ml_dtypes.bfloat16
    n, bb, r_sh = cfg["N"], cfg["BB"], cfg["R"]
    x = np.ascontiguousarray(x, dtype=np.float32)
    V = np.ascontiguousarray(V, dtype=np.float32)
    alpha = np.ascontiguousarray(alpha, dtype=np.float32)

    a = dykstra_mask(alpha)
    Amask = a[:, None] * V                     # [n, n] f32
    Amask2 = np.concatenate([Amask, Amask], axis=0)
    s0, s1 = Amask2.strides
    # BfullT[c, m] = Amask2[n + m - c, c] = a[(m-c)%n] * V[(m-c)%n, c]
    BfullT = np.lib.stride_tricks.as_strided(
        Amask2[n:], shape=(n, n), strides=(s1 - s0, s0)
    )
    bts = [BfullT[:, r0 : r0 + r_sh].astype(bf16) for r0 in range(0, n, r_sh)]
    del Amask, Amask2, BfullT

    xc = x.astype(bf16)
    xts = [
        np.ascontiguousarray(xc[b0 : b0 + bb].T) for b0 in range(0, x.shape[0], bb)
    ]

    in_maps = []
    for ib in range(x.shape[0] // bb):
        for ir in range(n // r_sh):
            in_maps.append({"xt_in": xts[ib], "bt_in": bts[ir]})
    return in_maps


_nc_cache = None


def kernel(x, V, alpha):
    """Full-input, full-output entry point. Shards over 8 NeuronCores."""
    from concourse import bass_utils

    global _nc_cache
    if _nc_cache is None:
        _nc_cache = build_nc(CFG_FULL)
    nc = _nc_cache

    in_maps = host_prep(x, V, alpha, CFG_FULL)
    res = bass_utils.run_bass_kernel_spmd(nc, in_maps, core_ids=list(range(8)))
    kernel.last_results = res

    bb, r_sh = CFG_FULL["BB"], CFG_FULL["R"]
    out = np.empty((BATCH, N), np.float32)
    for core, rmap in enumerate(res.results):
        ib, ir = divmod(core, N // r_sh)
        out[bb * ib : bb * (ib + 1), r_sh * ir : r_sh * (ir + 1)] = rmap["out_d"]
    return out


# revision 10
# speedup vs baseline: 1.0180x; 1.0180x over previous
"""Trainium2 Bass kernel: topk-masked pseudo-diagonal linear layer.

Math:  a = dykstra_topk(alpha);  W[r,c] = a[(r-c)%n] * V[(r-c)%n, c];
       out = x @ W.T,   with n = 8192, x [1024, 8192], V [8192, 8192].

Strategy (8 NeuronCores, SPMD, no collectives):
  - 2D shard: 4-way over out-features r (R=2048/core) x 2-way over batch
    (BB=512/core).  Each core computes out[b0:b0+512, r0:r0+2048].
  - Host synthesizes the sharded W.T band directly (Dykstra projection is
    a tiny 8192-vector fixed-point loop; the mask is multiplied into V
    during the band gather) and ships it in bf16:
        B[c, j] = a[(r0+j-c)%n] * V[(r0+j-c)%n, c]      [n, R] bf16
    plus the per-core x.T slice in bf16.
  - Device is a pure dense matmul out = x @ B streaming B from HBM with
    x.T resident in SBUF.  bf16 operands stream the PE at 1 col/cycle
    (2.4 GHz warm) vs ~0.5 for fp32r, so the kernel is PE-bound at
    ~218 us/core with DMA (~44 MiB/core) fully hidden.
"""

import math
import numpy as np

# ---- problem constants (hardcoded; must match reference.py) ----
N = 8192
BATCH = 1024
KTOP = math.ceil((1.0 - 0.9) * N * N / N)  # 820
LR = 0.05
ITERS = 50

# ---- sharding / tiling config ----
CFG_FULL = dict(N=N, BB=512, R=2048, TN=512)


def build_nc(cfg=CFG_FULL):
    """Build + compile the single-core SPMD Bass program."""
    import concourse.bass as bass
    import concourse.tile as tile
    from concourse import bacc, mybir

    f32 = mybir.dt.float32
    bf16 = mybir.dt.bfloat16

    n, bb, r_sh, tn = cfg["N"], cfg["BB"], cfg["R"], cfg["TN"]
    rhalf = r_sh // 2
    nct = n // 128          # c-tiles (64)
    nbt = bb // 128         # b-tiles (4)
    nrt = rhalf // tn       # r-subtiles per half (2)
    assert nbt * nrt <= 8

    nc = bacc.Bacc(
        "TRN2", target_bir_lowering=False, debug=False, enable_asserts=False
    )
    xt_in = nc.dram_tensor("xt_in", [n, bb], bf16, kind="ExternalInput").ap()
    bt_in = nc.dram_tensor("bt_in", [n, r_sh], bf16, kind="ExternalInput").ap()
    out_d = nc.dram_tensor("out_d", [bb, r_sh], f32, kind="ExternalOutput").ap()

    with tile.TileContext(nc) as tc:
        with (
            tc.tile_pool(name="wm", bufs=1) as wmp,
            tc.tile_pool(name="xt", bufs=1) as xtp,
            tc.tile_pool(name="bt", bufs=8) as btp,
            tc.tile_pool(name="ps", bufs=8, space=bass.MemorySpace.PSUM) as psp,
            tc.tile_pool(name="st", bufs=8) as stp,
        ):
            # ---- HAM warmup: ~6us of tiny matmuls on garbage data keeps the
            # PE busy through the cold-start window so the real matmuls run
            # at 2.4 GHz from the first tile, and overlaps the DMA prologue.
            wm_t = wmp.tile([128, 128], bf16)
            nc.vector.memset(wm_t[:], 0.0)
            wm_ps = psp.tile([128, 64], f32, tag="mm", name="wm_ps")
            for _ in range(72):
                nc.tensor.matmul(
                    wm_ps[:], wm_t[:], wm_t[:, 0:64], start=True, stop=True
                )

            # ---- first B tiles pre-issued so the main loop's first matmul
            # has data as early as possible ----
            rings = [nc.sync, nc.scalar]
            pre_bt = []
            for ct in range(4):
                b_t = btp.tile([128, rhalf], bf16, tag="bt")
                rings[ct % 2].dma_start(
                    b_t[:], bt_in[128 * ct : 128 * (ct + 1), 0:rhalf]
                )
                pre_bt.append(b_t)

            # ---- resident xT load, graduated chunks so the first c-tiles
            # land quickly (gpsimd ring; B tiles use sync/scalar) ----
            xt_sb = xtp.tile([128, nct, bb], bf16)
            edges = [0, 2, 4, 8, 16, 28, 40, 52, 64]
            for ch in range(8):
                c0, c1 = edges[ch], edges[ch + 1]
                nc.gpsimd.dma_start(
                    xt_sb[:, c0:c1, :],
                    xt_in[128 * c0 : 128 * c1, :].rearrange(
                        "(ct p) b -> p ct b", p=128
                    ),
                )

            # ---- main loop: stream B tiles, matmul into 8 PSUM banks ----
            for h in range(2):
                ps_tiles = [
                    psp.tile([128, tn], f32, tag="mm", name=f"ps_{h}_{i}")
                    for i in range(nbt * nrt)
                ]
                for ct in range(nct):
                    if h == 0 and ct < 4:
                        b_t = pre_bt[ct]
                    else:
                        b_t = btp.tile([128, rhalf], bf16, tag="bt")
                        rings[ct % 2].dma_start(
                            b_t[:],
                            bt_in[
                                128 * ct : 128 * (ct + 1),
                                rhalf * h : rhalf * (h + 1),
                            ],
                        )
                    for bt in range(nbt):
                        lhsT = xt_sb[:, ct, 128 * bt : 128 * (bt + 1)]
                        for rt in range(nrt):
                            nc.tensor.matmul(
                                ps_tiles[bt * nrt + rt][:],
                                lhsT,
                                b_t[:, tn * rt : tn * (rt + 1)],
                                start=(ct == 0),
                                stop=(ct == nct - 1),
                            )
                for bt in range(nbt):
                    for rt in range(nrt):
                        i = bt * nrt + rt
                        st_t = stp.tile([128, tn], f32, tag="st")
                        nc.vector.tensor_copy(st_t[:], ps_tiles[i][:])
                        [nc.gpsimd, nc.sync][i % 2].dma_start(
                            out_d[
                                128 * bt : 128 * (bt + 1),
                                rhalf * h + tn * rt : rhalf * h + tn * (rt + 1),
                            ],
                            st_t[:],
                        )
    nc.compile()
    return nc


# ---------------- host-side prep / gather ----------------

def dykstra_mask(alpha, k=KTOP, l=LR, num_iter=ITERS):
    """Euclidean projection of alpha/l onto {0 <= p <= 1, sum(p) = k}."""
    v = alpha.astype(np.float64) / l
    n = v.shape[0]
    p = np.zeros_like(v)
    q = np.zeros_like(v)
    for _ in range(num_iter):
        t = v + p
        y = t + (k - t.sum()) / n
        p = t - y
        v = np.clip(y + q, 0.0, 1.0)
        q = y + q - v
    return v.astype(np.float32)


def host_prep(x, V, alpha, cfg=CFG_FULL):
    """Build the 8 per-core input maps. Core id = ib*4 + ir."""
    import ml_dtypes

    bf16 = ml_dtypes.bfloat16
    n, bb, r_sh = cfg["N"], cfg["BB"], cfg["R"]
    x = np.ascontiguousarray(x, dtype=np.float32)
    V = np.ascontiguousarray(V, dtype=np.float32)
    alpha = np.ascontiguousarray(alpha, dtype=np.float32)

    a = dykstra_mask(alpha)
    Amask = a[:, None] * V                     # [n, n] f32
    Amask2 = np.concatenate([Amask, Amask], axis=0)
    s0, s1 = Amask2.strides
    # BfullT[c, m] = Amask2[n + m - c, c] = a[(m-c)%n] * V[(m-c)%n, c]
    BfullT = np.lib.stride_tricks.as_strided(
        Amask2[n:], shape=(n, n), strides=(s1 - s0, s0)
    )
    bts = [BfullT[:, r0 : r0 + r_sh].astype(bf16) for r0 in range(0, n, r_sh)]
    del Amask, Amask2, BfullT

    xc = x.astype(bf16)
    xts = [
        np.ascontiguousarray(xc[b0 : b0 + bb].T) for b0 in range(0, x.shape[0], bb)
    ]

    in_maps = []
    for ib in range(x.shape[0] // bb):
        for ir in range(n // r_sh):
            in_maps.append({"xt_in": xts[ib], "bt_in": bts[ir]})
    return in_maps


_nc_cache = None


def kernel(x, V, alpha):
    """Full-input, full-output entry point. Shards over 8 NeuronCores."""
    from concourse import bass_utils

    global _nc_cache
    if _nc_cache is None:
        _nc_cache = build_nc(CFG_FULL)
    nc = _nc_cache

    in_maps = host_prep(x, V, alpha, CFG_FULL)
    res = bass_utils.run_bass_kernel_spmd(nc, in_maps, core_ids=list(range(8)))
    kernel.last_results = res

    bb, r_sh = CFG_FULL["BB"], CFG_FULL["R"]
    out = np.empty((BATCH, N), np.float32)
    for core, rmap in enumerate(res.results):
        ib, ir = divmod(core, N // r_sh)
        out[bb * ib : bb * (ib + 1), r_sh * ir : r_sh * (ir + 1)] = rmap["out_d"]
    return out


# revision 15
# speedup vs baseline: 1.0566x; 1.0379x over previous
"""Trainium2 Bass kernel: topk-masked pseudo-diagonal linear layer.

Math:  a = dykstra_topk(alpha);  W[r,c] = a[(r-c)%n] * V[(r-c)%n, c];
       out = x @ W.T,   with n = 8192, x [1024, 8192], V [8192, 8192].

Strategy (8 NeuronCores, SPMD, no collectives):
  - 2D shard: 4-way over out-features r (R=2048/core) x 2-way over batch
    (BB=512/core).  Each core computes out[b0:b0+512, r0:r0+2048].
  - Host synthesizes the sharded W.T band directly (Dykstra projection is
    a tiny 8192-vector fixed-point loop; the mask is multiplied into V
    during the band gather) and ships it in bf16:
        B[c, j] = a[(r0+j-c)%n] * V[(r0+j-c)%n, c]      [n, R] bf16
    plus the per-core x.T slice in bf16.
  - Device is a pure dense matmul out = x @ B streaming B from HBM with
    x.T resident in SBUF.  bf16 operands stream the PE at 1 col/cycle
    (2.4 GHz warm) vs ~0.5 for fp32r, so the kernel is PE-bound at
    ~218 us/core with DMA (~44 MiB/core) fully hidden.
"""

import math
import numpy as np

# ---- problem constants (hardcoded; must match reference.py) ----
N = 8192
BATCH = 1024
KTOP = math.ceil((1.0 - 0.9) * N * N / N)  # 820
LR = 0.05
ITERS = 50

# ---- sharding / tiling config ----
CFG_FULL = dict(N=N, BB=512, R=2048, TN=512)


def build_nc(cfg=CFG_FULL):
    """Build + compile the single-core SPMD Bass program."""
    import concourse.bass as bass
    import concourse.tile as tile
    from concourse import bacc, mybir

    f32 = mybir.dt.float32
    bf16 = mybir.dt.bfloat16

    n, bb, r_sh, tn = cfg["N"], cfg["BB"], cfg["R"], cfg["TN"]
    rhalf = r_sh // 2
    nct = n // 128          # c-tiles (64)
    nbt = bb // 128         # b-tiles (4)
    nrt = rhalf // tn       # r-subtiles per half (2)
    assert nbt * nrt <= 8

    nc = bacc.Bacc(
        "TRN2", target_bir_lowering=False, debug=False, enable_asserts=False
    )
    xt_in = nc.dram_tensor("xt_in", [n, bb], bf16, kind="ExternalInput").ap()
    bt_in = nc.dram_tensor("bt_in", [n, r_sh], bf16, kind="ExternalInput").ap()
    out_d = nc.dram_tensor("out_d", [bb, r_sh], bf16, kind="ExternalOutput").ap()
    # out viewed as [p, bt, f] so one DMA covers all four 128-row blocks
    out_r = out_d.rearrange("(bt p) f -> p bt f", p=128)

    with tile.TileContext(nc) as tc:
        with (
            tc.tile_pool(name="wm", bufs=1) as wmp,
            tc.tile_pool(name="xt", bufs=1) as xtp,
            tc.tile_pool(name="bt", bufs=8) as btp,
            tc.tile_pool(name="ps", bufs=8, space=bass.MemorySpace.PSUM) as psp,
            tc.tile_pool(name="st", bufs=4) as stp,
        ):
            # ---- HAM warmup: ~6us of tiny matmuls on garbage data keeps the
            # PE busy through the cold-start window so the real matmuls run
            # at 2.4 GHz from the first tile, and overlaps the DMA prologue.
            wm_t = wmp.tile([128, 128], bf16)
            nc.vector.memset(wm_t[:], 0.0)
            wm_ps = psp.tile([128, 64], f32, tag="mm", name="wm_ps")
            for _ in range(104):
                nc.tensor.matmul(
                    wm_ps[:], wm_t[:], wm_t[:, 0:64], start=True, stop=True
                )

            # ---- critical first tiles on the fast HWDGE rings: the first
            # x.T c-tile and the first half of B[ct=0] gate the first real
            # matmul, so keep them small (128 KB) and first in queue ----
            rings = [nc.sync, nc.scalar]
            xt_sb = xtp.tile([128, nct, bb], bf16)
            nc.sync.dma_start(
                xt_sb[:, 0:1, :],
                xt_in[0:128, :].rearrange("(ct p) b -> p ct b", p=128),
            )
            b0_t = btp.tile([128, rhalf], bf16, tag="bt")
            nc.scalar.dma_start(b0_t[:, 0:tn], bt_in[0:128, 0:tn])
            nc.sync.dma_start(b0_t[:, tn:rhalf], bt_in[0:128, tn:rhalf])
            pre_bt = [b0_t]
            for ct in range(1, 4):
                b_t = btp.tile([128, rhalf], bf16, tag="bt")
                rings[ct % 2].dma_start(
                    b_t[:], bt_in[128 * ct : 128 * (ct + 1), 0:rhalf]
                )
                pre_bt.append(b_t)

            # ---- rest of the resident xT load, graduated chunks so early
            # c-tiles land quickly (gpsimd ring; B tiles use sync/scalar) ----
            edges = [1, 2, 4, 8, 16, 28, 40, 52, 64]
            for ch in range(8):
                c0, c1 = edges[ch], edges[ch + 1]
                nc.gpsimd.dma_start(
                    xt_sb[:, c0:c1, :],
                    xt_in[128 * c0 : 128 * c1, :].rearrange(
                        "(ct p) b -> p ct b", p=128
                    ),
                )

            # ---- main loop: stream B tiles, matmul into 8 PSUM banks ----
            for h in range(2):
                ps_tiles = [
                    psp.tile([128, tn], f32, tag="mm", name=f"ps_{h}_{i}")
                    for i in range(nbt * nrt)
                ]
                for ct in range(nct):
                    if h == 0 and ct < 4:
                        b_t = pre_bt[ct]
                    else:
                        b_t = btp.tile([128, rhalf], bf16, tag="bt")
                        rings[ct % 2].dma_start(
                            b_t[:],
                            bt_in[
                                128 * ct : 128 * (ct + 1),
                                rhalf * h : rhalf * (h + 1),
                            ],
                        )
                    for bt in range(nbt):
                        lhsT = xt_sb[:, ct, 128 * bt : 128 * (bt + 1)]
                        for rt in range(nrt):
                            nc.tensor.matmul(
                                ps_tiles[bt * nrt + rt][:],
                                lhsT,
                                b_t[:, tn * rt : tn * (rt + 1)],
                                start=(ct == 0),
                                stop=(ct == nct - 1),
                            )
                # drain PSUM: one staging tile + one DMA per r-subtile, with
                # copies split across Vector and Scalar (different banks can
                # be read in parallel); 4 big out-DMAs total instead of 16
                # small ones, so their ~2us completion latencies don't chain.
                for rt in range(nrt):
                    st_t = stp.tile([128, nbt, tn], bf16, tag="st")
                    for bt in range(nbt):
                        i = bt * nrt + rt
                        if bt % 2 == 0:
                            nc.vector.tensor_copy(st_t[:, bt, :], ps_tiles[i][:])
                        else:
                            nc.scalar.copy(st_t[:, bt, :], ps_tiles[i][:])
                    [nc.gpsimd, nc.sync][rt].dma_start(
                        out_r[:, :, rhalf * h + tn * rt : rhalf * h + tn * (rt + 1)],
                        st_t[:],
                    )
    nc.compile()
    return nc


# ---------------- host-side prep / gather ----------------

def dykstra_mask(alpha, k=KTOP, l=LR, num_iter=ITERS):
    """Euclidean projection of alpha/l onto {0 <= p <= 1, sum(p) = k}."""
    v = alpha.astype(np.float64) / l
    n = v.shape[0]
    p = np.zeros_like(v)
    q = np.zeros_like(v)
    for _ in range(num_iter):
        t = v + p
        y = t + (k - t.sum()) / n
        p = t - y
        v = np.clip(y + q, 0.0, 1.0)
        q = y + q - v
    return v.astype(np.float32)


def host_prep(x, V, alpha, cfg=CFG_FULL):
    """Build the 8 per-core input maps. Core id = ib*4 + ir."""
    import ml_dtypes

    bf16 = ml_dtypes.bfloat16
    n, bb, r_sh = cfg["N"], cfg["BB"], cfg["R"]
    x = np.ascontiguousarray(x, dtype=np.float32)
    V = np.ascontiguousarray(V, dtype=np.float32)
    alpha = np.ascontiguousarray(alpha, dtype=np.float32)

    a = dykstra_mask(alpha)
    Amask = a[:, None] * V                     # [n, n] f32
    Amask2 = np.concatenate([Amask, Amask], axis=0)
    s0, s1 = Amask2.strides
    # BfullT[c, m] = Amask2[n + m - c, c] = a[(m-c)%n] * V[(m-c)%n, c]
    BfullT = np.lib.stride_tricks.as_strided(
        Amask2[n:], shape=(n, n), strides=(s1 - s0, s0)
    )
    bts = [BfullT[:, r0 : r0 + r_sh].astype(bf16) for r0 in range(0, n, r_sh)]
    del Amask, Amask2, BfullT

    xc = x.astype(bf16)
    xts = [
        np.ascontiguousarray(xc[b0 : b0 + bb].T) for b0 in range(0, x.shape[0], bb)
    ]

    in_maps = []
    for ib in range(x.shape[0] // bb):
        for ir in range(n // r_sh):
            in_maps.append({"xt_in": xts[ib], "bt_in": bts[ir]})
    return in_maps


_nc_cache = None


def kernel(x, V, alpha):
    """Full-input, full-output entry point. Shards over 8 NeuronCores."""
    from concourse import bass_utils

    global _nc_cache
    if _nc_cache is None:
        _nc_cache = build_nc(CFG_FULL)
    nc = _nc_cache

    in_maps = host_prep(x, V, alpha, CFG_FULL)
    res = bass_utils.run_bass_kernel_spmd(nc, in_maps, core_ids=list(range(8)))
    kernel.last_results = res

    bb, r_sh = CFG_FULL["BB"], CFG_FULL["R"]
    out = np.empty((BATCH, N), np.float32)
    for core, rmap in enumerate(res.results):
        ib, ir = divmod(core, N // r_sh)
        out[bb * ib : bb * (ib + 1), r_sh * ir : r_sh * (ir + 1)] = rmap[
            "out_d"
        ].astype(np.float32)
    return out


# revision 16
# speedup vs baseline: 1.0577x; 1.0010x over previous
"""Trainium2 Bass kernel: topk-masked pseudo-diagonal linear layer.

Math:  a = dykstra_topk(alpha);  W[r,c] = a[(r-c)%n] * V[(r-c)%n, c];
       out = x @ W.T,   with n = 8192, x [1024, 8192], V [8192, 8192].

Strategy (8 NeuronCores, SPMD, no collectives):
  - 2D shard: 4-way over out-features r (R=2048/core) x 2-way over batch
    (BB=512/core).  Each core computes out[b0:b0+512, r0:r0+2048].
  - Host synthesizes the sharded W.T band directly (Dykstra projection is
    a tiny 8192-vector fixed-point loop; the mask is multiplied into V
    during the band gather) and ships it in bf16:
        B[c, j] = a[(r0+j-c)%n] * V[(r0+j-c)%n, c]      [n, R] bf16
    plus the per-core x.T slice in bf16.
  - Device is a pure dense matmul out = x @ B streaming B from HBM with
    x.T resident in SBUF.  bf16 operands stream the PE at 1 col/cycle
    (2.4 GHz warm) vs ~0.5 for fp32r, so the kernel is PE-bound at
    ~218 us/core with DMA (~44 MiB/core) fully hidden.
"""

import math
import numpy as np

# ---- problem constants (hardcoded; must match reference.py) ----
N = 8192
BATCH = 1024
KTOP = math.ceil((1.0 - 0.9) * N * N / N)  # 820
LR = 0.05
ITERS = 50

# ---- sharding / tiling config ----
CFG_FULL = dict(N=N, BB=512, R=2048, TN=512)


def build_nc(cfg=CFG_FULL):
    """Build + compile the single-core SPMD Bass program."""
    import concourse.bass as bass
    import concourse.tile as tile
    from concourse import bacc, mybir

    f32 = mybir.dt.float32
    bf16 = mybir.dt.bfloat16

    n, bb, r_sh, tn = cfg["N"], cfg["BB"], cfg["R"], cfg["TN"]
    rhalf = r_sh // 2
    nct = n // 128          # c-tiles (64)
    nbt = bb // 128         # b-tiles (4)
    nrt = rhalf // tn       # r-subtiles per half (2)
    assert nbt * nrt <= 8

    nc = bacc.Bacc(
        "TRN2", target_bir_lowering=False, debug=False, enable_asserts=False
    )
    xt_in = nc.dram_tensor("xt_in", [n, bb], bf16, kind="ExternalInput").ap()
    bt_in = nc.dram_tensor("bt_in", [n, r_sh], bf16, kind="ExternalInput").ap()
    out_d = nc.dram_tensor("out_d", [bb, r_sh], bf16, kind="ExternalOutput").ap()
    # out viewed as [p, bt, f] so one DMA covers all four 128-row blocks
    out_r = out_d.rearrange("(bt p) f -> p bt f", p=128)

    with tile.TileContext(nc) as tc:
        with (
            tc.tile_pool(name="wm", bufs=1) as wmp,
            tc.tile_pool(name="xt", bufs=1) as xtp,
            tc.tile_pool(name="bt", bufs=8) as btp,
            tc.tile_pool(name="ps", bufs=8, space=bass.MemorySpace.PSUM) as psp,
            tc.tile_pool(name="st", bufs=4) as stp,
        ):
            # ---- HAM warmup: ~6us of tiny matmuls on garbage data keeps the
            # PE busy through the cold-start window so the real matmuls run
            # at 2.4 GHz from the first tile, and overlaps the DMA prologue.
            wm_t = wmp.tile([128, 128], bf16)
            nc.vector.memset(wm_t[:], 0.0)
            wm_ps = psp.tile([128, 64], f32, tag="mm", name="wm_ps")
            for _ in range(104):
                nc.tensor.matmul(
                    wm_ps[:], wm_t[:], wm_t[:, 0:64], start=True, stop=True
                )

            # ---- critical first tiles on the fast HWDGE rings: the first
            # x.T c-tile and the first half of B[ct=0] gate the first real
            # matmul, so keep them small (128 KB) and first in queue ----
            # (the Scalar ring's first DMA is delayed ~1.3us by the framework
            # ACT_TABLE_LOAD preamble, so both gating tiles go on Sync first)
            rings = [nc.sync, nc.scalar]
            xt_sb = xtp.tile([128, nct, bb], bf16)
            b0_t = btp.tile([128, rhalf], bf16, tag="bt")
            nc.sync.dma_start(b0_t[:, 0:tn], bt_in[0:128, 0:tn])
            nc.sync.dma_start(
                xt_sb[:, 0:1, :],
                xt_in[0:128, :].rearrange("(ct p) b -> p ct b", p=128),
            )
            nc.scalar.dma_start(b0_t[:, tn:rhalf], bt_in[0:128, tn:rhalf])
            pre_bt = [b0_t]
            for ct in range(1, 4):
                b_t = btp.tile([128, rhalf], bf16, tag="bt")
                rings[ct % 2].dma_start(
                    b_t[:], bt_in[128 * ct : 128 * (ct + 1), 0:rhalf]
                )
                pre_bt.append(b_t)

            # ---- rest of the resident xT load, graduated chunks so early
            # c-tiles land quickly (gpsimd ring; B tiles use sync/scalar) ----
            edges = [1, 2, 4, 8, 16, 28, 40, 52, 64]
            for ch in range(8):
                c0, c1 = edges[ch], edges[ch + 1]
                nc.gpsimd.dma_start(
                    xt_sb[:, c0:c1, :],
                    xt_in[128 * c0 : 128 * c1, :].rearrange(
                        "(ct p) b -> p ct b", p=128
                    ),
                )

            # ---- main loop: stream B tiles, matmul into 8 PSUM banks ----
            for h in range(2):
                ps_tiles = [
                    psp.tile([128, tn], f32, tag="mm", name=f"ps_{h}_{i}")
                    for i in range(nbt * nrt)
                ]
                for ct in range(nct):
                    if h == 0 and ct < 4:
                        b_t = pre_bt[ct]
                    else:
                        b_t = btp.tile([128, rhalf], bf16, tag="bt")
                        rings[ct % 2].dma_start(
                            b_t[:],
                            bt_in[
                                128 * ct : 128 * (ct + 1),
                                rhalf * h : rhalf * (h + 1),
                            ],
                        )
                    for bt in range(nbt):
                        lhsT = xt_sb[:, ct, 128 * bt : 128 * (bt + 1)]
                        for rt in range(nrt):
                            nc.tensor.matmul(
                                ps_tiles[bt * nrt + rt][:],
                                lhsT,
                                b_t[:, tn * rt : tn * (rt + 1)],
                                start=(ct == 0),
                                stop=(ct == nct - 1),
                            )
                # drain PSUM: one staging tile + one DMA per r-subtile, with
                # copies split across Vector and Scalar (different banks can
                # be read in parallel); 4 big out-DMAs total instead of 16
                # small ones, so their ~2us completion latencies don't chain.
                for rt in range(nrt):
                    st_t = stp.tile([128, nbt, tn], bf16, tag="st")
                    for bt in range(nbt):
                        i = bt * nrt + rt
                        if bt % 2 == 0:
                            nc.vector.tensor_copy(st_t[:, bt, :], ps_tiles[i][:])
                        else:
                            nc.scalar.copy(st_t[:, bt, :], ps_tiles[i][:])
                    [nc.gpsimd, nc.sync][rt].dma_start(
                        out_r[:, :, rhalf * h + tn * rt : rhalf * h + tn * (rt + 1)],
                        st_t[:],
                    )
    nc.compile()
    return nc


# ---------------- host-side prep / gather ----------------

def dykstra_mask(alpha, k=KTOP, l=LR, num_iter=ITERS):
    """Euclidean projection of alpha/l onto {0 <= p <= 1, sum(p) = k}."""
    v = alpha.astype(np.float64) / l
    n = v.shape[0]
    p = np.zeros_like(v)
    q = np.zeros_like(v)
    for _ in range(num_iter):
        t = v + p
        y = t + (k - t.sum()) / n
        p = t - y
        v = np.clip(y + q, 0.0, 1.0)
        q = y + q - v
    return v.astype(np.float32)


def host_prep(x, V, alpha, cfg=CFG_FULL):
    """Build the 8 per-core input maps. Core id = ib*4 + ir."""
    import ml_dtypes

    bf16 = ml_dtypes.bfloat16
    n, bb, r_sh = cfg["N"], cfg["BB"], cfg["R"]
    x = np.ascontiguousarray(x, dtype=np.float32)
    V = np.ascontiguousarray(V, dtype=np.float32)
    alpha = np.ascontiguousarray(alpha, dtype=np.float32)

    a = dykstra_mask(alpha)
    Amask = a[:, None] * V                     # [n, n] f32
    Amask2 = np.concatenate([Amask, Amask], axis=0)
    s0, s1 = Amask2.strides
    # BfullT[c, m] = Amask2[n + m - c, c] = a[(m-c)%n] * V[(m-c)%n, c]
    BfullT = np.lib.stride_tricks.as_strided(
        Amask2[n:], shape=(n, n), strides=(s1 - s0, s0)
    )
    bts = [BfullT[:, r0 : r0 + r_sh].astype(bf16) for r0 in range(0, n, r_sh)]
    del Amask, Amask2, BfullT

    xc = x.astype(bf16)
    xts = [
        np.ascontiguousarray(xc[b0 : b0 + bb].T) for b0 in range(0, x.shape[0], bb)
    ]

    in_maps = []
    for ib in range(x.shape[0] // bb):
        for ir in range(n // r_sh):
            in_maps.append({"xt_in": xts[ib], "bt_in": bts[ir]})
    return in_maps


_nc_cache = None


def kernel(x, V, alpha):
    """Full-input, full-output entry point. Shards over 8 NeuronCores."""
    from concourse import bass_utils

    global _nc_cache
    if _nc_cache is None:
        _nc_cache = build_nc(CFG_FULL)
    nc = _nc_cache

    in_maps = host_prep(x, V, alpha, CFG_FULL)
    res = bass_utils.run_bass_kernel_spmd(nc, in_maps, core_ids=list(range(8)))
    kernel.last_results = res

    bb, r_sh = CFG_FULL["BB"], CFG_FULL["R"]
    out = np.empty((BATCH, N), np.float32)
    for core, rmap in enumerate(res.results):
        ib, ir = divmod(core, N // r_sh)
        out[bb * ib : bb * (ib + 1), r_sh * ir : r_sh * (ir + 1)] = rmap[
            "out_d"
        ].astype(np.float32)
    return out


# revision 19
# speedup vs baseline: 1.0674x; 1.0092x over previous
"""Trainium2 Bass kernel: topk-masked pseudo-diagonal linear layer.

Math:  a = dykstra_topk(alpha);  W[r,c] = a[(r-c)%n] * V[(r-c)%n, c];
       out = x @ W.T,   with n = 8192, x [1024, 8192], V [8192, 8192].

Strategy (8 NeuronCores, SPMD, no collectives):
  - 2D shard: 4-way over out-features r (R=2048/core) x 2-way over batch
    (BB=512/core).  Each core computes out[b0:b0+512, r0:r0+2048].
  - Host synthesizes the sharded W.T band directly (Dykstra projection is
    a tiny 8192-vector fixed-point loop; the mask is multiplied into V
    during the band gather) and ships it in bf16:
        B[c, j] = a[(r0+j-c)%n] * V[(r0+j-c)%n, c]      [n, R] bf16
    plus the per-core x.T slice in bf16.
  - Device is a pure dense matmul out = x @ B streaming B from HBM with
    x.T resident in SBUF.  bf16 operands stream the PE at 1 col/cycle
    (2.4 GHz warm) vs ~0.5 for fp32r, so the kernel is PE-bound at
    ~218 us/core with DMA (~44 MiB/core) fully hidden.
"""

import math
import numpy as np

# ---- problem constants (hardcoded; must match reference.py) ----
N = 8192
BATCH = 1024
KTOP = math.ceil((1.0 - 0.9) * N * N / N)  # 820
LR = 0.05
ITERS = 50

# ---- sharding / tiling config ----
CFG_FULL = dict(N=N, BB=512, R=2048, TN=512)


def build_nc(cfg=CFG_FULL):
    """Build + compile the single-core SPMD Bass program."""
    import concourse.bass as bass
    import concourse.tile as tile
    from concourse import bacc, mybir

    f32 = mybir.dt.float32
    bf16 = mybir.dt.bfloat16

    n, bb, r_sh, tn = cfg["N"], cfg["BB"], cfg["R"], cfg["TN"]
    rhalf = r_sh // 2
    nct = n // 128          # c-tiles (64)
    nbt = bb // 128         # b-tiles (4)
    nrt = rhalf // tn       # r-subtiles per half (2)
    assert nbt * nrt <= 8

    nc = bacc.Bacc(
        "TRN2", target_bir_lowering=False, debug=False, enable_asserts=False
    )
    xt_in = nc.dram_tensor("xt_in", [n, bb], bf16, kind="ExternalInput").ap()
    bt_in = nc.dram_tensor("bt_in", [n, r_sh], bf16, kind="ExternalInput").ap()
    # out scratch in staging order [h, rt, p, bt, tn] so each of the 4
    # out-DMAs is fully contiguous on the DRAM side (4KB lines, full rate);
    # the host un-permutes with a cheap numpy transpose.
    out_d = nc.dram_tensor(
        "out_d", [2, nrt, 128, nbt, tn], bf16, kind="ExternalOutput"
    ).ap()

    with tile.TileContext(nc) as tc:
        with (
            tc.tile_pool(name="wm", bufs=1) as wmp,
            tc.tile_pool(name="xt", bufs=1) as xtp,
            tc.tile_pool(name="bt", bufs=8) as btp,
            tc.tile_pool(name="ps", bufs=8, space=bass.MemorySpace.PSUM) as psp,
            tc.tile_pool(name="st", bufs=4) as stp,
        ):
            # ---- HAM warmup: ~6us of tiny matmuls on garbage data keeps the
            # PE busy through the cold-start window so the real matmuls run
            # at 2.4 GHz from the first tile, and overlaps the DMA prologue.
            wm_t = wmp.tile([128, 128], bf16)
            nc.vector.memset(wm_t[:], 0.0)
            wm_ps = psp.tile([128, 64], f32, tag="mm", name="wm_ps")
            for _ in range(104):
                nc.tensor.matmul(
                    wm_ps[:], wm_t[:], wm_t[:, 0:64], start=True, stop=True
                )

            # ---- critical first tiles on the fast HWDGE rings: the first
            # x.T c-tile and the first half of B[ct=0] gate the first real
            # matmul, so keep them small (128 KB) and first in queue ----
            # (the Scalar ring's first DMA is delayed ~1.3us by the framework
            # ACT_TABLE_LOAD preamble, so both gating tiles go on Sync first)
            rings = [nc.sync, nc.scalar]
            xt_sb = xtp.tile([128, nct, bb], bf16)
            b0_t = btp.tile([128, rhalf], bf16, tag="bt")
            nc.sync.dma_start(b0_t[:, 0:tn], bt_in[0:128, 0:tn])
            nc.sync.dma_start(
                xt_sb[:, 0:1, :],
                xt_in[0:128, :].rearrange("(ct p) b -> p ct b", p=128),
            )
            nc.scalar.dma_start(b0_t[:, tn:rhalf], bt_in[0:128, tn:rhalf])
            pre_bt = [b0_t]
            for ct in range(1, 4):
                b_t = btp.tile([128, rhalf], bf16, tag="bt")
                rings[ct % 2].dma_start(
                    b_t[:], bt_in[128 * ct : 128 * (ct + 1), 0:rhalf]
                )
                pre_bt.append(b_t)

            # ---- rest of the resident xT load, graduated chunks so early
            # c-tiles land quickly (gpsimd ring; B tiles use sync/scalar) ----
            edges = [1, 2, 4, 8, 16, 28, 40, 52, 64]
            for ch in range(8):
                c0, c1 = edges[ch], edges[ch + 1]
                nc.gpsimd.dma_start(
                    xt_sb[:, c0:c1, :],
                    xt_in[128 * c0 : 128 * c1, :].rearrange(
                        "(ct p) b -> p ct b", p=128
                    ),
                )

            # ---- main loop: stream B tiles, matmul into 8 PSUM banks ----
            for h in range(2):
                ps_tiles = [
                    psp.tile([128, tn], f32, tag="mm", name=f"ps_{h}_{i}")
                    for i in range(nbt * nrt)
                ]
                for ct in range(nct):
                    if h == 0 and ct < 4:
                        b_t = pre_bt[ct]
                    else:
                        b_t = btp.tile([128, rhalf], bf16, tag="bt")
                        rings[ct % 2].dma_start(
                            b_t[:],
                            bt_in[
                                128 * ct : 128 * (ct + 1),
                                rhalf * h : rhalf * (h + 1),
                            ],
                        )
                    for bt in range(nbt):
                        lhsT = xt_sb[:, ct, 128 * bt : 128 * (bt + 1)]
                        for rt in range(nrt):
                            nc.tensor.matmul(
                                ps_tiles[bt * nrt + rt][:],
                                lhsT,
                                b_t[:, tn * rt : tn * (rt + 1)],
                                start=(ct == 0),
                                stop=(ct == nct - 1),
                            )
                # drain PSUM: one staging tile + one DMA per r-subtile, with
                # copies split across Vector and Scalar (different banks can
                # be read in parallel); 4 big out-DMAs total instead of 16
                # small ones, so their ~2us completion latencies don't chain.
                for rt in range(nrt):
                    st_t = stp.tile([128, nbt, tn], bf16, tag="st")
                    for bt in range(nbt):
                        i = bt * nrt + rt
                        if bt % 2 == 0:
                            nc.vector.tensor_copy(st_t[:, bt, :], ps_tiles[i][:])
                        else:
                            nc.scalar.copy(st_t[:, bt, :], ps_tiles[i][:])
                    [nc.sync, nc.scalar][rt].dma_start(out_d[h, rt], st_t[:])
    nc.compile()
    return nc


# ---------------- host-side prep / gather ----------------

def dykstra_mask(alpha, k=KTOP, l=LR, num_iter=ITERS):
    """Euclidean projection of alpha/l onto {0 <= p <= 1, sum(p) = k}."""
    v = alpha.astype(np.float64) / l
    n = v.shape[0]
    p = np.zeros_like(v)
    q = np.zeros_like(v)
    for _ in range(num_iter):
        t = v + p
        y = t + (k - t.sum()) / n
        p = t - y
        v = np.clip(y + q, 0.0, 1.0)
        q = y + q - v
    return v.astype(np.float32)


def host_prep(x, V, alpha, cfg=CFG_FULL):
    """Build the 8 per-core input maps. Core id = ib*4 + ir."""
    import ml_dtypes

    bf16 = ml_dtypes.bfloat16
    n, bb, r_sh = cfg["N"], cfg["BB"], cfg["R"]
    x = np.ascontiguousarray(x, dtype=np.float32)
    V = np.ascontiguousarray(V, dtype=np.float32)
    alpha = np.ascontiguousarray(alpha, dtype=np.float32)

    a = dykstra_mask(alpha)
    Amask = a[:, None] * V                     # [n, n] f32
    Amask2 = np.concatenate([Amask, Amask], axis=0)
    s0, s1 = Amask2.strides
    # BfullT[c, m] = Amask2[n + m - c, c] = a[(m-c)%n] * V[(m-c)%n, c]
    BfullT = np.lib.stride_tricks.as_strided(
        Amask2[n:], shape=(n, n), strides=(s1 - s0, s0)
    )
    bts = [BfullT[:, r0 : r0 + r_sh].astype(bf16) for r0 in range(0, n, r_sh)]
    del Amask, Amask2, BfullT

    xc = x.astype(bf16)
    xts = [
        np.ascontiguousarray(xc[b0 : b0 + bb].T) for b0 in range(0, x.shape[0], bb)
    ]

    in_maps = []
    for ib in range(x.shape[0] // bb):
        for ir in range(n // r_sh):
            in_maps.append({"xt_in": xts[ib], "bt_in": bts[ir]})
    return in_maps


_nc_cache = None


def kernel(x, V, alpha):
    """Full-input, full-output entry point. Shards over 8 NeuronCores."""
    from concourse import bass_utils

    global _nc_cache
    if _nc_cache is None:
        _nc_cache = build_nc(CFG_FULL)
    nc = _nc_cache

    in_maps = host_prep(x, V, alpha, CFG_FULL)
    res = bass_utils.run_bass_kernel_spmd(nc, in_maps, core_ids=list(range(8)))
    kernel.last_results = res

    bb, r_sh = CFG_FULL["BB"], CFG_FULL["R"]
    out = np.empty((BATCH, N), np.float32)
    for core, rmap in enumerate(res.results):
        ib, ir = divmod(core, N // r_sh)
        # [h, rt, p, bt, tn] -> [bt*128+p, h*1024 + rt*512 + i]
        blk = rmap["out_d"].transpose(3, 2, 0, 1, 4).reshape(bb, r_sh)
        out[bb * ib : bb * (ib + 1), r_sh * ir : r_sh * (ir + 1)] = blk.astype(
            np.float32
        )
    return out
